# revision 31
# baseline (speedup 1.0000x reference)
# Causal self-attention (B=2, S=2048, D=1024, H=16) on 8 TRN2 NeuronCores.
#
# Sharding: core = (batch b, head-group hg) with 4 heads per core — data
# parallel on B (cores 0-3 = batch 0, cores 4-7 = batch 1), tensor parallel
# on heads within each batch group. Per core:
#   1. 32 warmup matmuls run while the input DMAs land (HAM 4/8 -> 8/8),
#      then ALL Q^T/K^T projections (heads pair-packed on partitions
#      0-63/64-127) and V projected directly to NATURAL [token, dim] layout
#      with bias and a ones-column folded in (K=1 matmul) so the ctx
#      matmul's row 64 accumulates the softmax denominator.
#   2. attention per query piece, LARGEST FIRST (512-token chunks 3,2,1
#      then the 256-token halves of chunk 0 to shrink the serial tail):
#      scores^T with keys on partitions; the two heads of a pair run as
#      CONCURRENT row-group-packed K=64 matmuls into the two banks of one
#      [128, 2w] psum tile; ONE exp ACT op covers both heads (split in two
#      for diagonal tiles); diagonal masking via one strided tri-multiply.
#   3. normalize: per-pair denominators at partitions 0/64, one [65,w]
#      DVE reciprocal per pair; broadcast via K=1 matmuls; the whole norm
#      + AllGather of piece k is emitted two k-tiles into piece k+1 so the
#      reciprocal latency never stalls the PE.
#   4. out-projection of piece k emitted ~3 pieces later, so collectives
#      (~20us each) are never on the PE critical path; per-piece output
#      DMA. Collective staging DMAs ride the idle GpSimd queue.
# Host side shards/pre-transposes inputs and concatenates the 8 output
# column-slices; no host arithmetic beyond dtype casts and transposes.

import numpy as np
import ml_dtypes

import concourse.bass as bass
import concourse.mybir as mybir
import concourse.tile as tile
from concourse import bacc
from concourse.bass_utils import run_bass_kernel_spmd
from concourse.masks import make_upper_triangular

F32 = mybir.dt.float32
BF16 = mybir.dt.bfloat16

B, S, D, H, HD = 2, 2048, 1024, 16, 64
HG = 4                 # heads per core
DG = HG * HD           # 256 qkv cols per head-group
NCORES = 8
KT = 128               # key tile (partition dim of scoresT)
QC = 512               # projection chunk
NKT = S // KT          # 16 key tiles
NQC = S // QC          # 4 projection chunks
SM_SCALE = 1.0 / 8.0   # 1/sqrt(HD)
KC = D // 128          # 8 contraction chunks for the projections

# attention pieces (lo, w), processed in this order; chunk 0 is split so
# the final gather+out_proj tail is as short as possible
PIECES = [(3 * QC, QC), (2 * QC, QC), (1 * QC, QC), (0, 256), (256, 256)]

_NP = {BF16: ml_dtypes.bfloat16, F32: np.float32}

LAST_RESULTS = None    # BassKernelResults of the most recent kernel() call
_NC_CACHE = {}


def _build_nc():
    nc = bacc.Bacc(
        trn_type="TRN2",
        target_bir_lowering=False,
        debug=False,
        num_devices=NCORES,
    )

    xT = nc.declare_dram_parameter("xT", [D, S], BF16, isOutput=False)
    wqkv = nc.declare_dram_parameter("wqkv", [D, 3 * DG], BF16, isOutput=False)
    bqkv = nc.declare_dram_parameter("bqkv", [128, 4], F32, isOutput=False)
    bv = nc.declare_dram_parameter("bv", [1, DG], BF16, isOutput=False)
    wout = nc.declare_dram_parameter("wout", [D, DG], BF16, isOutput=False)
    bout = nc.declare_dram_parameter("bout", [128, 2], F32, isOutput=False)
    outT = nc.declare_dram_parameter("outT", [DG, S], F32, isOutput=True)

    with tile.TileContext(nc) as tc:
        with tc.tile_pool(name="persist", bufs=1) as ps:
            # ---- constants ----
            tri = ps.tile([128, 128], F32, tag="tri")
            make_upper_triangular(nc, tri, val=1.0, diag=True)
            tri2 = ps.tile([128, 2, 128], BF16, tag="tri2")
            nc.vector.tensor_copy(tri2[:, 0, :], tri)
            nc.vector.tensor_copy(tri2[:, 1, :], tri)
            ones_row = ps.tile([1, 128], BF16, tag="ones_row")
            nc.vector.memset(ones_row, 1.0)
            ones2 = ps.tile([128, 64], BF16, tag="ones2")
            nc.vector.memset(ones2[0:1, :], 1.0)
            nc.vector.memset(ones2[64:65, :], 1.0)

            # ---- persistent SBUF tensors ----
            xT_sb = ps.tile([128, KC, S], BF16, tag="xT_sb")
            wqkv_sb = ps.tile([128, KC, 3 * DG], BF16, tag="wqkv_sb")
            bqkv_sb = ps.tile([128, 4], F32, tag="bqkv_sb")
            bv_sb = ps.tile([1, DG], BF16, tag="bv_sb")
            qk_sb = ps.tile([128, 4, S], BF16, tag="qk_sb")       # Q^T,K^T
            vnat_sb = ps.tile([128, NKT, HG, HD + 1], BF16, tag="vnat_sb")
            ctx_sb = ps.tile([128, 2, S], BF16, tag="ctx_sb")     # normalized
            ctxg_sb = ps.tile([128, D // 128, S], BF16, tag="ctxg_sb")
            wout_sb = ps.tile([128, KC, DG], BF16, tag="wout_sb")
            bout_sb = ps.tile([128, 2], F32, tag="bout_sb")
            outT_sb = ps.tile([128, 2, S], F32, tag="outT_sb")

            nc.vector.memset(vnat_sb, 1.0)

            # denominator staging, double-buffered across pieces; rows at
            # partitions 0 and 64 only (legal engine base partitions)
            dens_sb = [[ps.tile([65, QC], F32, tag=f"dens{b_}{m}",
                                name=f"dens{b_}{m}") for m in range(2)]
                       for b_ in range(2)]
            recip_sb = [[ps.tile([65, QC], BF16, tag=f"recip{b_}{m}",
                                 name=f"recip{b_}{m}") for m in range(2)]
                        for b_ in range(2)]
            for b_ in range(2):
                nc.vector.memset(dens_sb[b_][0], 1.0)
                nc.vector.memset(dens_sb[b_][1], 1.0)

            # ---- load inputs ----
            xT_r = xT.rearrange("(c p) s -> c p s", p=128)
            wqkv_r = wqkv.rearrange("(c p) m -> c p m", p=128)
            wout_r = wout.rearrange("(c p) m -> c p m", p=128)
            for c in range(KC):
                nc.sync.dma_start(out=wqkv_sb[:, c, :], in_=wqkv_r[c])
            nc.sync.dma_start(out=bqkv_sb, in_=bqkv[:])
            nc.sync.dma_start(out=bv_sb, in_=bv[:])
            for n0 in range(NQC):
                for c in range(KC):
                    nc.sync.dma_start(
                        out=xT_sb[:, c, n0 * QC:(n0 + 1) * QC],
                        in_=xT_r[c][:, n0 * QC:(n0 + 1) * QC])
            for c in range(KC):
                nc.sync.dma_start(out=wout_sb[:, c, :], in_=wout_r[c])
            nc.sync.dma_start(out=bout_sb, in_=bout[:])

            with tc.tile_pool(name="dram", bufs=1, space="DRAM") as dram:
                cc_in = [dram.tile([DG, w], BF16, tag=f"cc_in{k}",
                                   name=f"cc_in{k}")
                         for k, (lo, w) in enumerate(PIECES)]
                cc_out = [dram.tile([D, w], BF16, tag=f"cc_out{k}",
                                    name=f"cc_out{k}")
                          for k, (lo, w) in enumerate(PIECES)]

                def proj_chunk(n):
                    q0 = n * QC
                    for m in range(4):
                        pt = pp.tile([128, QC], F32, tag="pp")
                        for c in range(KC):
                            nc.tensor.matmul(
                                pt,
                                lhsT=wqkv_sb[:, c, m * 128:(m + 1) * 128],
                                rhs=xT_sb[:, c, q0:q0 + QC],
                                start=(c == 0),
                                stop=(c == KC - 1),
                            )
                        nc.vector.tensor_scalar_add(
                            qk_sb[:, m, q0:q0 + QC], pt, bqkv_sb[:, m:m + 1])
                    # V natural: tokens on partitions; bias+ones via K=1 mm
                    for t in range(4 * n, 4 * n + 4):
                        vp = pp.tile([128, QC], F32, tag="pp")
                        vdat = vp[:, 0:DG]
                        nc.tensor.matmul(
                            vdat, lhsT=ones_row, rhs=bv_sb[:],
                            start=True, stop=False,
                        )
                        for c in range(KC):
                            nc.tensor.matmul(
                                vdat,
                                lhsT=xT_sb[:, c, t * KT:(t + 1) * KT],
                                rhs=wqkv_sb[:, c, 2 * DG:3 * DG],
                                start=False,
                                stop=(c == KC - 1),
                            )
                        nc.vector.tensor_copy(
                            vnat_sb[:, t, :, 0:HD],
                            vdat.rearrange("p (h d) -> p h d", d=HD))

                raws_all = {}

                def attn_core(k, hook=None):
                    lo, w = PIECES[k]
                    raws_t = [None, None]
                    n_kt = (lo + w) // KT
                    hook_at = min(2, n_kt - 1)
                    for mh in range(2):
                        hA, hB = 2 * mh, 2 * mh + 1
                        cxA = cxp.tile([HD + 1, w], F32, tag="cx")
                        cxB = cxp.tile([HD + 1, w], F32, tag="cx")
                        for i in range(n_kt):
                            tsh = KT * i - lo
                            t0 = max(tsh, 0)
                            # head B's half always at offset QC so the two
                            # concurrent row-packed matmuls never share a
                            # psum bank (w<QC halves would collide)
                            sc = scp.tile([128, 2 * QC], F32, tag="sc")
                            at = asb.tile([128, 2 * QC], BF16, tag="at")
                            nc.tensor.matmul(
                                sc[:, t0:w],
                                lhsT=qk_sb[0:64, 2 + mh,
                                           i * KT:(i + 1) * KT],
                                rhs=qk_sb[0:64, mh, lo + t0:lo + w],
                                start=True, stop=True,
                            )
                            nc.tensor.matmul(
                                sc[:, QC + t0:QC + w],
                                lhsT=qk_sb[64:128, 2 + mh,
                                           i * KT:(i + 1) * KT],
                                rhs=qk_sb[64:128, mh, lo + t0:lo + w],
                                start=True, stop=True,
                            )
                            if t0 or w != QC:
                                nc.scalar.activation(
                                    at[:, t0:w], sc[:, t0:w],
                                    mybir.ActivationFunctionType.Exp,
                                    scale=SM_SCALE)
                                nc.scalar.activation(
                                    at[:, QC + t0:QC + w],
                                    sc[:, QC + t0:QC + w],
                                    mybir.ActivationFunctionType.Exp,
                                    scale=SM_SCALE)
                            else:
                                nc.scalar.activation(
                                    at[:], sc[:],
                                    mybir.ActivationFunctionType.Exp,
                                    scale=SM_SCALE)
                            if tsh >= 0:   # diagonal: mask k > q, both heads
                                atm = at[:].rearrange(
                                    "p (b q) -> p b q", b=2)[:, :, t0:t0 + KT]
                                nc.vector.tensor_mul(atm, atm, tri2[:])
                            nc.tensor.matmul(
                                cxA[:, t0:w],
                                lhsT=vnat_sb[:, i, hA, :],
                                rhs=at[:, t0:w],
                                start=(i == 0),
                                stop=(i == n_kt - 1),
                            )
                            nc.tensor.matmul(
                                cxB[:, t0:w],
                                lhsT=vnat_sb[:, i, hB, :],
                                rhs=at[:, QC + t0:QC + w],
                                start=(i == 0),
                                stop=(i == n_kt - 1),
                            )
                            if mh == 0 and i == hook_at and hook is not None:
                                hook()
                        raw = rsb.tile([128, QC], BF16, tag="raw", bufs=5)
                        for po, cx in ((0, cxA), (64, cxB)):
                            nc.vector.tensor_copy(
                                raw[po:po + HD, 0:w], cx[0:HD, :])
                            nc.vector.tensor_copy(
                                dens_sb[k % 2][mh][po:po + 1, 0:w],
                                cx[HD:HD + 1, :])
                        raws_t[mh] = raw
                    raws_all[k] = raws_t

                def norm_recip(k):
                    lo, w = PIECES[k]
                    for mh in range(2):
                        with nc.allow_low_precision(
                                reason="softmax denominator broadcast"):
                            nc.vector.reciprocal(
                                recip_sb[k % 2][mh][0:65, 0:w],
                                dens_sb[k % 2][mh][0:65, 0:w])

                def norm_apply(k):
                    lo, w = PIECES[k]
                    for mh in range(2):
                        bcp = pp.tile([128, QC], F32, tag="pp")
                        nc.tensor.matmul(
                            bcp[0:64, 0:w], lhsT=ones2[0:1, :],
                            rhs=recip_sb[k % 2][mh][0:1, 0:w],
                            start=True, stop=True)
                        nc.tensor.matmul(
                            bcp[64:128, 0:w], lhsT=ones2[64:65, :],
                            rhs=recip_sb[k % 2][mh][64:65, 0:w],
                            start=True, stop=True)
                        nc.vector.tensor_mul(
                            ctx_sb[:, mh, lo:lo + w],
                            raws_all[k][mh][:, 0:w], bcp[:, 0:w])

                def gather_piece(k):
                    lo, w = PIECES[k]
                    cc_in_r = cc_in[k].rearrange("(c p) s -> c p s", p=128)
                    for c in range(2):
                        nc.gpsimd.dma_start(
                            out=cc_in_r[c], in_=ctx_sb[:, c, lo:lo + w])
                    nc.gpsimd.collective_compute(
                        "AllGather",
                        mybir.AluOpType.bypass,
                        replica_groups=[[0, 1, 2, 3], [4, 5, 6, 7]],
                        ins=[cc_in[k][:].opt()],
                        outs=[cc_out[k][:].opt()],
                    )
                    cc_out_r = cc_out[k].rearrange("(c p) s -> c p s", p=128)
                    for c in range(D // 128):
                        nc.gpsimd.dma_start(
                            out=ctxg_sb[:, c, lo:lo + w], in_=cc_out_r[c])

                outT_r = outT.rearrange("(c p) s -> c p s", p=128)

                def out_proj_piece(k):
                    lo, w = PIECES[k]
                    for mo in range(2):
                        pt = pp.tile([128, QC], F32, tag="pp")
                        for c in range(KC):
                            nc.tensor.matmul(
                                pt[:, 0:w],
                                lhsT=wout_sb[:, c, mo * 128:(mo + 1) * 128],
                                rhs=ctxg_sb[:, c, lo:lo + w],
                                start=(c == 0),
                                stop=(c == KC - 1),
                            )
                        nc.vector.tensor_scalar_add(
                            outT_sb[:, mo, lo:lo + w], pt[:, 0:w],
                            bout_sb[:, mo:mo + 1])
                        nc.sync.dma_start(
                            out=outT_r[mo][:, lo:lo + w],
                            in_=outT_sb[:, mo, lo:lo + w])

                with tc.tile_pool(name="proj_ps", bufs=2, space="PSUM") as pp, \
                     tc.tile_pool(name="sc_ps", bufs=2, space="PSUM") as scp, \
                     tc.tile_pool(name="ctx_ps", bufs=2, space="PSUM") as cxp, \
                     tc.tile_pool(name="attn_sb", bufs=3) as asb, \
                     tc.tile_pool(name="raw_sb", bufs=5) as rsb:
                    # warm the PE (HAM 4/8 -> 8/8) while input DMAs land
                    wp = pp.tile([128, QC], F32, tag="pp")
                    for _ in range(32):
                        nc.tensor.matmul(
                            wp[:, 0:128], lhsT=ones_row, rhs=ones_row,
                            start=True, stop=True)
                    for n in range(NQC):
                        proj_chunk(n)

                    def mk_hook(kk):
                        def h():
                            norm_apply(kk)
                            gather_piece(kk)
                        return h

                    npc = len(PIECES)
                    for idx in range(npc):
                        if idx > 0:
                            norm_recip(idx - 1)
                        attn_core(idx,
                                  hook=mk_hook(idx - 1) if idx > 0 else None)
                        if idx > 2:
                            out_proj_piece(idx - 3)
                    norm_recip(npc - 1)
                    norm_apply(npc - 1)
                    gather_piece(npc - 1)
                    out_proj_piece(npc - 3)
                    out_proj_piece(npc - 2)
                    out_proj_piece(npc - 1)

    nc.compile()
    return nc


def get_nc():
    if "nc" not in _NC_CACHE:
        _NC_CACHE["nc"] = _build_nc()
    return _NC_CACHE["nc"]


def make_in_maps(x, w_qkv, b_qkv, w_out, b_out):
    x = np.asarray(x, np.float32)
    w_qkv = np.asarray(w_qkv, np.float32)
    b_qkv = np.asarray(b_qkv, np.float32)
    w_out = np.asarray(w_out, np.float32)
    b_out = np.asarray(b_out, np.float32)

    bf16 = ml_dtypes.bfloat16
    xT = [np.ascontiguousarray(x[b].T).astype(bf16) for b in range(B)]
    in_maps = []
    for core in range(NCORES):
        b, hg = core // HG, core % HG
        sl = slice(hg * DG, (hg + 1) * DG)
        wq = w_qkv[:, sl]
        wk = w_qkv[:, D + hg * DG:D + (hg + 1) * DG]
        wv = w_qkv[:, 2 * D + hg * DG:2 * D + (hg + 1) * DG]
        wqkv_s = np.ascontiguousarray(
            np.concatenate([wq, wk, wv], axis=1)).astype(bf16)
        bqk = np.concatenate(
            [b_qkv[sl], b_qkv[D + hg * DG:D + (hg + 1) * DG]])
        bvv = b_qkv[2 * D + hg * DG:2 * D + (hg + 1) * DG]
        in_maps.append({
            "xT": xT[b],
            "wqkv": wqkv_s,
            "bqkv": np.ascontiguousarray(
                bqk.reshape(4, 128).T).astype(np.float32),
            "bv": np.ascontiguousarray(bvv.reshape(1, DG)).astype(bf16),
            "wout": np.ascontiguousarray(w_out[:, sl]).astype(bf16),
            "bout": np.ascontiguousarray(
                b_out[sl].reshape(2, 128).T).astype(np.float32),
        })
    return in_maps


def assemble_output(results):
    out = np.empty((B, S, D), np.float32)
    for core in range(NCORES):
        b, hg = core // HG, core % HG
        out[b, :, hg * DG:(hg + 1) * DG] = results[core]["outT"].T
    return out


def kernel(x, w_qkv, b_qkv, w_out, b_out):
    global LAST_RESULTS
    in_maps = make_in_maps(x, w_qkv, b_qkv, w_out, b_out)
    nc = get_nc()
    res = run_bass_kernel_spmd(nc, in_maps, list(range(NCORES)))
    LAST_RESULTS = res
    return assemble_output(res.results)


# revision 32
# speedup vs baseline: 1.1142x; 1.1142x over previous
# Causal self-attention (B=2, S=2048, D=1024, H=16) on 8 TRN2 NeuronCores.
#
# Sharding: core = (batch b, head-group hg) with 4 heads per core — data
# parallel on B (cores 0-3 = batch 0, cores 4-7 = batch 1), tensor parallel
# on heads within each batch group. Per core:
#   1. 32 warmup matmuls run while the input DMAs land (HAM 4/8 -> 8/8),
#      then ALL Q^T/K^T projections (heads pair-packed on partitions
#      0-63/64-127) and V projected directly to NATURAL [token, dim] layout
#      with bias and a ones-column folded in (K=1 matmul) so the ctx
#      matmul's row 64 accumulates the softmax denominator.
#   2. attention per query piece, LARGEST FIRST (512-token chunks 3,2,1
#      then the 256-token halves of chunk 0 to shrink the serial tail):
#      scores^T with keys on partitions; the two heads of a pair run as
#      CONCURRENT row-group-packed K=64 matmuls into the two banks of one
#      [128, 2w] psum tile; ONE exp ACT op covers both heads (split in two
#      for diagonal tiles); diagonal masking via one strided tri-multiply.
#   3. normalize: per-pair denominators at partitions 0/64, one [65,w]
#      DVE reciprocal per pair; broadcast via K=1 matmuls; the whole norm
#      + AllGather of piece k is emitted two k-tiles into piece k+1 so the
#      reciprocal latency never stalls the PE.
#   4. out-projection of piece k emitted ~3 pieces later, so collectives
#      (~20us each) are never on the PE critical path; per-piece output
#      DMA. Collective staging DMAs ride the idle GpSimd queue.
# Host side shards/pre-transposes inputs and concatenates the 8 output
# column-slices; no host arithmetic beyond dtype casts and transposes.

import numpy as np
import ml_dtypes

import concourse.bass as bass
import concourse.mybir as mybir
import concourse.tile as tile
from concourse import bacc
from concourse.bass_utils import run_bass_kernel_spmd
from concourse.masks import make_upper_triangular

F32 = mybir.dt.float32
BF16 = mybir.dt.bfloat16

B, S, D, H, HD = 2, 2048, 1024, 16, 64
HG = 4                 # heads per core
DG = HG * HD           # 256 qkv cols per head-group
NCORES = 8
KT = 128               # key tile (partition dim of scoresT)
QC = 512               # projection chunk
NKT = S // KT          # 16 key tiles
NQC = S // QC          # 4 projection chunks
SM_SCALE = 1.0 / 8.0   # 1/sqrt(HD)
KC = D // 128          # 8 contraction chunks for the projections

# attention pieces (lo, w), processed in this order; chunk 0 is split so
# the final gather+out_proj tail is as short as possible
PIECES = [(0, QC), (2 * QC, QC), (3 * QC, QC), (QC, 256), (QC + 256, 256)]
# proj chunks emitted after each attention piece (keeps ACT busy early
# while satisfying attn(piece) -> proj(chunks <= piece end) deps)
PROJ_AFTER = {0: (1, 2), 1: (3,)}

_NP = {BF16: ml_dtypes.bfloat16, F32: np.float32}

LAST_RESULTS = None    # BassKernelResults of the most recent kernel() call
_NC_CACHE = {}


def _build_nc():
    nc = bacc.Bacc(
        trn_type="TRN2",
        target_bir_lowering=False,
        debug=False,
        num_devices=NCORES,
    )

    xT = nc.declare_dram_parameter("xT", [D, S], BF16, isOutput=False)
    wqkv = nc.declare_dram_parameter("wqkv", [D, 3 * DG], BF16, isOutput=False)
    bqkv = nc.declare_dram_parameter("bqkv", [128, 4], F32, isOutput=False)
    bv = nc.declare_dram_parameter("bv", [1, DG], BF16, isOutput=False)
    wout = nc.declare_dram_parameter("wout", [D, DG], BF16, isOutput=False)
    bout = nc.declare_dram_parameter("bout", [128, 2], F32, isOutput=False)
    outT = nc.declare_dram_parameter("outT", [DG, S], F32, isOutput=True)

    with tile.TileContext(nc) as tc:
        with tc.tile_pool(name="persist", bufs=1) as ps:
            # ---- constants (warmup operands first) ----
            ones_row = ps.tile([1, 128], BF16, tag="ones_row")
            nc.vector.memset(ones_row, 1.0)
            warm_rhs = ps.tile([1, QC], BF16, tag="warm_rhs")
            nc.vector.memset(warm_rhs, 1.0)
            tri = ps.tile([128, 128], F32, tag="tri")
            make_upper_triangular(nc, tri, val=1.0, diag=True)
            tri2 = ps.tile([128, 2, 128], BF16, tag="tri2")
            nc.vector.tensor_copy(tri2[:, 0, :], tri)
            nc.vector.tensor_copy(tri2[:, 1, :], tri)
            ones2 = ps.tile([128, 64], BF16, tag="ones2")
            nc.vector.memset(ones2[0:1, :], 1.0)
            nc.vector.memset(ones2[64:65, :], 1.0)

            # ---- persistent SBUF tensors ----
            xT_sb = ps.tile([128, KC, S], BF16, tag="xT_sb")
            wqkv_sb = ps.tile([128, KC, 3 * DG], BF16, tag="wqkv_sb")
            bqkv_sb = ps.tile([128, 4], F32, tag="bqkv_sb")
            bv_sb = ps.tile([1, DG], BF16, tag="bv_sb")
            qk_sb = ps.tile([128, 4, S], BF16, tag="qk_sb")       # Q^T,K^T
            vnat_sb = ps.tile([128, NKT, HG, HD + 1], BF16, tag="vnat_sb")
            ctx_sb = ps.tile([128, 2, S], BF16, tag="ctx_sb")     # normalized
            ctxg_sb = ps.tile([128, D // 128, S], BF16, tag="ctxg_sb")
            wout_sb = ps.tile([128, KC, DG], BF16, tag="wout_sb")
            bout_sb = ps.tile([128, 2], F32, tag="bout_sb")
            outT_sb = ps.tile([128, 2, S], F32, tag="outT_sb")

            nc.vector.memset(vnat_sb, 1.0)

            # denominator staging, double-buffered across pieces; rows at
            # partitions 0 and 64 only (legal engine base partitions)
            dens_sb = [[ps.tile([65, QC], F32, tag=f"dens{b_}{m}",
                                name=f"dens{b_}{m}") for m in range(2)]
                       for b_ in range(2)]
            recip_sb = [[ps.tile([65, QC], BF16, tag=f"recip{b_}{m}",
                                 name=f"recip{b_}{m}") for m in range(2)]
                        for b_ in range(2)]
            for b_ in range(2):
                nc.vector.memset(dens_sb[b_][0], 1.0)
                nc.vector.memset(dens_sb[b_][1], 1.0)

            # ---- load inputs ----
            xT_r = xT.rearrange("(c p) s -> c p s", p=128)
            wqkv_r = wqkv.rearrange("(c p) m -> c p m", p=128)
            wout_r = wout.rearrange("(c p) m -> c p m", p=128)
            for c in range(KC):
                nc.sync.dma_start(out=wqkv_sb[:, c, :], in_=wqkv_r[c])
            nc.sync.dma_start(out=bqkv_sb, in_=bqkv[:])
            nc.sync.dma_start(out=bv_sb, in_=bv[:])
            for n0 in range(NQC):
                for c in range(KC):
                    nc.sync.dma_start(
                        out=xT_sb[:, c, n0 * QC:(n0 + 1) * QC],
                        in_=xT_r[c][:, n0 * QC:(n0 + 1) * QC])
            for c in range(KC):
                nc.sync.dma_start(out=wout_sb[:, c, :], in_=wout_r[c])
            nc.sync.dma_start(out=bout_sb, in_=bout[:])

            with tc.tile_pool(name="dram", bufs=1, space="DRAM") as dram:
                cc_in = [dram.tile([DG, w], BF16, tag=f"cc_in{k}",
                                   name=f"cc_in{k}")
                         for k, (lo, w) in enumerate(PIECES)]
                cc_out = [dram.tile([D, w], BF16, tag=f"cc_out{k}",
                                    name=f"cc_out{k}")
                          for k, (lo, w) in enumerate(PIECES)]

                def proj_chunk(n):
                    q0 = n * QC
                    for m in range(4):
                        pt = pp.tile([128, QC], F32, tag="pp")
                        for c in range(KC):
                            nc.tensor.matmul(
                                pt,
                                lhsT=wqkv_sb[:, c, m * 128:(m + 1) * 128],
                                rhs=xT_sb[:, c, q0:q0 + QC],
                                start=(c == 0),
                                stop=(c == KC - 1),
                            )
                        nc.vector.tensor_scalar_add(
                            qk_sb[:, m, q0:q0 + QC], pt, bqkv_sb[:, m:m + 1])
                    # V natural: tokens on partitions; bias+ones via K=1 mm
                    for t in range(4 * n, 4 * n + 4):
                        vp = pp.tile([128, QC], F32, tag="pp")
                        vdat = vp[:, 0:DG]
                        nc.tensor.matmul(
                            vdat, lhsT=ones_row, rhs=bv_sb[:],
                            start=True, stop=False,
                        )
                        for c in range(KC):
                            nc.tensor.matmul(
                                vdat,
                                lhsT=xT_sb[:, c, t * KT:(t + 1) * KT],
                                rhs=wqkv_sb[:, c, 2 * DG:3 * DG],
                                start=False,
                                stop=(c == KC - 1),
                            )
                        nc.vector.tensor_copy(
                            vnat_sb[:, t, :, 0:HD],
                            vdat.rearrange("p (h d) -> p h d", d=HD))

                raws_all = {}

                def attn_core(k, hook=None):
                    lo, w = PIECES[k]
                    raws_t = [None, None]
                    n_kt = (lo + w) // KT
                    hook_at = min(2, n_kt - 1)
                    for mh in range(2):
                        hA, hB = 2 * mh, 2 * mh + 1
                        cxA = cxp.tile([HD + 1, w], F32, tag="cx")
                        cxB = cxp.tile([HD + 1, w], F32, tag="cx")
                        for i in range(n_kt):
                            tsh = KT * i - lo
                            t0 = max(tsh, 0)
                            # head B's half always at offset QC so the two
                            # concurrent row-packed matmuls never share a
                            # psum bank (w<QC halves would collide)
                            sc = scp.tile([128, 2 * QC], F32, tag="sc")
                            at = asb.tile([128, 2 * QC], BF16, tag="at")
                            nc.tensor.matmul(
                                sc[:, t0:w],
                                lhsT=qk_sb[0:64, 2 + mh,
                                           i * KT:(i + 1) * KT],
                                rhs=qk_sb[0:64, mh, lo + t0:lo + w],
                                start=True, stop=True,
                            )
                            nc.tensor.matmul(
                                sc[:, QC + t0:QC + w],
                                lhsT=qk_sb[64:128, 2 + mh,
                                           i * KT:(i + 1) * KT],
                                rhs=qk_sb[64:128, mh, lo + t0:lo + w],
                                start=True, stop=True,
                            )
                            if t0 or w != QC:
                                nc.scalar.activation(
                                    at[:, t0:w], sc[:, t0:w],
                                    mybir.ActivationFunctionType.Exp,
                                    scale=SM_SCALE)
                                nc.scalar.activation(
                                    at[:, QC + t0:QC + w],
                                    sc[:, QC + t0:QC + w],
                                    mybir.ActivationFunctionType.Exp,
                                    scale=SM_SCALE)
                            else:
                                nc.scalar.activation(
                                    at[:], sc[:],
                                    mybir.ActivationFunctionType.Exp,
                                    scale=SM_SCALE)
                            if tsh >= 0:   # diagonal: mask k > q, both heads
                                atm = at[:].rearrange(
                                    "p (b q) -> p b q", b=2)[:, :, t0:t0 + KT]
                                nc.vector.tensor_mul(atm, atm, tri2[:])
                            nc.tensor.matmul(
                                cxA[:, t0:w],
                                lhsT=vnat_sb[:, i, hA, :],
                                rhs=at[:, t0:w],
                                start=(i == 0),
                                stop=(i == n_kt - 1),
                            )
                            nc.tensor.matmul(
                                cxB[:, t0:w],
                                lhsT=vnat_sb[:, i, hB, :],
                                rhs=at[:, QC + t0:QC + w],
                                start=(i == 0),
                                stop=(i == n_kt - 1),
                            )
                            if mh == 0 and i == hook_at and hook is not None:
                                hook()
                        raw = rsb.tile([128, QC], BF16, tag="raw", bufs=5)
                        for po, cx in ((0, cxA), (64, cxB)):
                            nc.vector.tensor_copy(
                                raw[po:po + HD, 0:w], cx[0:HD, :])
                            nc.vector.tensor_copy(
                                dens_sb[k % 2][mh][po:po + 1, 0:w],
                                cx[HD:HD + 1, :])
                        raws_t[mh] = raw
                    raws_all[k] = raws_t

                def norm_recip(k):
                    lo, w = PIECES[k]
                    for mh in range(2):
                        with nc.allow_low_precision(
                                reason="softmax denominator broadcast"):
                            nc.vector.reciprocal(
                                recip_sb[k % 2][mh][0:65, 0:w],
                                dens_sb[k % 2][mh][0:65, 0:w])

                def norm_apply(k):
                    lo, w = PIECES[k]
                    for mh in range(2):
                        bcp = pp.tile([128, QC], F32, tag="pp")
                        nc.tensor.matmul(
                            bcp[0:64, 0:w], lhsT=ones2[0:1, :],
                            rhs=recip_sb[k % 2][mh][0:1, 0:w],
                            start=True, stop=True)
                        nc.tensor.matmul(
                            bcp[64:128, 0:w], lhsT=ones2[64:65, :],
                            rhs=recip_sb[k % 2][mh][64:65, 0:w],
                            start=True, stop=True)
                        nc.vector.tensor_mul(
                            ctx_sb[:, mh, lo:lo + w],
                            raws_all[k][mh][:, 0:w], bcp[:, 0:w])

                def gather_piece(k):
                    lo, w = PIECES[k]
                    cc_in_r = cc_in[k].rearrange("(c p) s -> c p s", p=128)
                    for c in range(2):
                        nc.gpsimd.dma_start(
                            out=cc_in_r[c], in_=ctx_sb[:, c, lo:lo + w])
                    nc.gpsimd.collective_compute(
                        "AllGather",
                        mybir.AluOpType.bypass,
                        replica_groups=[[0, 1, 2, 3], [4, 5, 6, 7]],
                        ins=[cc_in[k][:].opt()],
                        outs=[cc_out[k][:].opt()],
                    )
                    cc_out_r = cc_out[k].rearrange("(c p) s -> c p s", p=128)
                    for c in range(D // 128):
                        nc.gpsimd.dma_start(
                            out=ctxg_sb[:, c, lo:lo + w], in_=cc_out_r[c])

                outT_r = outT.rearrange("(c p) s -> c p s", p=128)

                def out_proj_piece(k):
                    lo, w = PIECES[k]
                    for mo in range(2):
                        pt = pp.tile([128, QC], F32, tag="pp")
                        for c in range(KC):
                            nc.tensor.matmul(
                                pt[:, 0:w],
                                lhsT=wout_sb[:, c, mo * 128:(mo + 1) * 128],
                                rhs=ctxg_sb[:, c, lo:lo + w],
                                start=(c == 0),
                                stop=(c == KC - 1),
                            )
                        nc.vector.tensor_scalar_add(
                            outT_sb[:, mo, lo:lo + w], pt[:, 0:w],
                            bout_sb[:, mo:mo + 1])
                        nc.sync.dma_start(
                            out=outT_r[mo][:, lo:lo + w],
                            in_=outT_sb[:, mo, lo:lo + w])

                with tc.tile_pool(name="proj_ps", bufs=2, space="PSUM") as pp, \
                     tc.tile_pool(name="sc_ps", bufs=2, space="PSUM") as scp, \
                     tc.tile_pool(name="ctx_ps", bufs=2, space="PSUM") as cxp, \
                     tc.tile_pool(name="attn_sb", bufs=3) as asb, \
                     tc.tile_pool(name="raw_sb", bufs=5) as rsb:
                    # warm the PE (HAM 4/8 -> 8/8) while input DMAs land
                    wp = pp.tile([128, QC], F32, tag="pp")
                    for _ in range(40):
                        nc.tensor.matmul(
                            wp, lhsT=ones_row, rhs=warm_rhs,
                            start=True, stop=True)
                    proj_chunk(0)

                    def mk_hook(kk):
                        def h():
                            norm_apply(kk)
                            gather_piece(kk)
                        return h

                    npc = len(PIECES)
                    for idx in range(npc):
                        if idx > 0:
                            norm_recip(idx - 1)
                        attn_core(idx,
                                  hook=mk_hook(idx - 1) if idx > 0 else None)
                        for pn in PROJ_AFTER.get(idx, ()):
                            proj_chunk(pn)
                        if idx > 2:
                            out_proj_piece(idx - 3)
                    norm_recip(npc - 1)
                    norm_apply(npc - 1)
                    gather_piece(npc - 1)
                    out_proj_piece(npc - 3)
                    out_proj_piece(npc - 2)
                    out_proj_piece(npc - 1)

    nc.compile()
    return nc


def get_nc():
    if "nc" not in _NC_CACHE:
        _NC_CACHE["nc"] = _build_nc()
    return _NC_CACHE["nc"]


def make_in_maps(x, w_qkv, b_qkv, w_out, b_out):
    x = np.asarray(x, np.float32)
    w_qkv = np.asarray(w_qkv, np.float32)
    b_qkv = np.asarray(b_qkv, np.float32)
    w_out = np.asarray(w_out, np.float32)
    b_out = np.asarray(b_out, np.float32)

    bf16 = ml_dtypes.bfloat16
    xT = [np.ascontiguousarray(x[b].T).astype(bf16) for b in range(B)]
    in_maps = []
    for core in range(NCORES):
        b, hg = core // HG, core % HG
        sl = slice(hg * DG, (hg + 1) * DG)
        wq = w_qkv[:, sl]
        wk = w_qkv[:, D + hg * DG:D + (hg + 1) * DG]
        wv = w_qkv[:, 2 * D + hg * DG:2 * D + (hg + 1) * DG]
        wqkv_s = np.ascontiguousarray(
            np.concatenate([wq, wk, wv], axis=1)).astype(bf16)
        bqk = np.concatenate(
            [b_qkv[sl], b_qkv[D + hg * DG:D + (hg + 1) * DG]])
        bvv = b_qkv[2 * D + hg * DG:2 * D + (hg + 1) * DG]
        in_maps.append({
            "xT": xT[b],
            "wqkv": wqkv_s,
            "bqkv": np.ascontiguousarray(
                bqk.reshape(4, 128).T).astype(np.float32),
            "bv": np.ascontiguousarray(bvv.reshape(1, DG)).astype(bf16),
            "wout": np.ascontiguousarray(w_out[:, sl]).astype(bf16),
            "bout": np.ascontiguousarray(
                b_out[sl].reshape(2, 128).T).astype(np.float32),
        })
    return in_maps


def assemble_output(results):
    out = np.empty((B, S, D), np.float32)
    for core in range(NCORES):
        b, hg = core // HG, core % HG
        out[b, :, hg * DG:(hg + 1) * DG] = results[core]["outT"].T
    return out


def kernel(x, w_qkv, b_qkv, w_out, b_out):
    global LAST_RESULTS
    in_maps = make_in_maps(x, w_qkv, b_qkv, w_out, b_out)
    nc = get_nc()
    res = run_bass_kernel_spmd(nc, in_maps, list(range(NCORES)))
    LAST_RESULTS = res
    return assemble_output(res.results)
